# revision 1
# baseline (speedup 1.0000x reference)
"""Causal self-attention (B=4, T=2048, C=2048, H=16) on 8 trn2 NeuronCores.

Sharding: tensor-parallel over heads — 2 heads per core. Every core gets the
full (pre-transposed) activation xT, its 2 heads' slice of Wqkv columns and
Wproj rows, computes a full [B*T, C] partial output, and the host sums the 8
partials (the "all-reduce after output projection" done host-side).

Per-core dataflow (all matmuls on PE, fp32r for x/W precision, fp16 for the
attention-probability path):
  xT tiles --DMA--> QKV proj -> Q^T,K^T [d,t] (f32r) + V [t,d] (fp16)
  S = Q^T.T @ K^T chunks (PSUM f32) -> +causal mask -> exp (ACT, accum denom)
  P (fp16) -> normalize by 1/denom (DVE) -> PE-transpose -> P^T (fp16)
  y^T = sum_k V_k^T-block @ P^T-block (PSUM f32) -> f32r
  out_partial = y^T.T @ Wproj-rows (accumulate 2 head-chunks) -> DMA out
"""
import numpy as np

B, T, C = 4, 2048, 2048
H, HD = 16, 128
N_CORES = 8
HPC = H // N_CORES          # heads per core = 2
SCALE = float(1.0 / np.sqrt(HD))
NEG = -1e9

# "fp16": all matmul operands fp16 (1 cyc/row on PE).
# "fp32r": x/W path in fp32r (TF32-like, 2 passes -> 2x slower, ~2.5x more accurate).
MM_DT = "fp16"

_CACHE = {}


def _build_nc():
    import concourse.bass as bass
    from concourse import bacc
    import concourse.tile as tile
    import concourse.mybir as mybir
    from concourse.masks import make_identity, make_causal_mask
    from contextlib import ExitStack

    f32 = mybir.dt.float32
    f32r = mybir.dt.float32r
    f16 = mybir.dt.float16
    wdt = f16 if MM_DT == "fp16" else f32r
    in_dt = f32 if MM_DT == "fp32r" else f16
    Exp = mybir.ActivationFunctionType.Exp
    AX = mybir.AxisListType.X

    nc = bacc.Bacc("TRN2", target_bir_lowering=False, debug=False,
                   enable_asserts=True, num_devices=N_CORES)

    # Inputs (per-core shards prepared on host)
    xT = nc.dram_tensor("xt", [C, B * T], in_dt, kind="ExternalInput").ap()
    wqkv = nc.dram_tensor("wqkv", [C, 6 * HD], in_dt, kind="ExternalInput").ap()
    wproj = nc.dram_tensor("wproj", [HPC * HD, C], in_dt, kind="ExternalInput").ap()
    out = nc.dram_tensor("out", [B * T, C], f32, kind="ExternalOutput").ap()

    # DRAM views: c-chunked weights
    wqkv_v = wqkv.rearrange("(cc p) (jj d) -> p cc jj d", p=128, d=HD)  # [128,16,6,128]
    wproj_v = wproj.rearrange("(jh p) c -> p jh c", p=128)              # [128,2,2048]

    NCC = C // 128        # 16 contraction chunks
    NTCH = T // 512       # 4 t-chunks per batch

    with tile.TileContext(nc) as tc, ExitStack() as ctx:
        const = ctx.enter_context(tc.tile_pool(name="const", bufs=1))
        wpool = ctx.enter_context(tc.tile_pool(name="w", bufs=1))
        xtp = ctx.enter_context(tc.tile_pool(name="xt", bufs=2))
        qkvp = ctx.enter_context(tc.tile_pool(name="qkv", bufs=2))
        
        dnp = ctx.enter_context(tc.tile_pool(name="dn", bufs=4))
        rp = ctx.enter_context(tc.tile_pool(name="r", bufs=2))
        ptp = ctx.enter_context(tc.tile_pool(name="pt", bufs=2))
        ytp = ctx.enter_context(tc.tile_pool(name="yt", bufs=2))
        op = ctx.enter_context(tc.tile_pool(name="o", bufs=6))
        psA = ctx.enter_context(tc.tile_pool(name="psA", bufs=3, space="PSUM"))
        psV = ctx.enter_context(tc.tile_pool(name="psV", bufs=2, space="PSUM"))
        psD = ctx.enter_context(tc.tile_pool(name="psD", bufs=2, space="PSUM"))
        psT = ctx.enter_context(tc.tile_pool(name="psT", bufs=1, space="PSUM"))

        ident_f = const.tile([128, 128], f32)
        make_identity(nc, ident_f)
        ident_h = const.tile([128, 128], f16)
        nc.scalar.copy(ident_h, ident_f)
        # transposed-orientation causal mask: keep (partition=k_rel) <= (free=q_rel)
        triT = const.tile([128, 128], f32)
        nc.gpsimd.memset(triT, 0.0)
        nc.gpsimd.affine_select(
            out=triT, in_=triT, compare_op=mybir.AluOpType.is_ge, fill=NEG,
            base=0, pattern=[[1, 128]], channel_multiplier=-1)
        ones_col = const.tile([128, 1], f16)
        nc.vector.memset(ones_col, 1.0)
        ones_row = const.tile([1, 128], f16)
        nc.vector.memset(ones_row, 1.0)

        w_sb = wpool.tile([128, NCC, 6, HD], wdt)
        nc.sync.dma_start(w_sb, wqkv_v if MM_DT == "fp16" else wqkv_v.bitcast(f32r))
        wp_sb = wpool.tile([128, 2, C], wdt)
        nc.sync.dma_start(wp_sb, wproj_v if MM_DT == "fp16" else wproj_v.bitcast(f32r))

        def emit_qkv_chunk(b, tch, qkv_tiles):
            qt, kt, vt, v = qkv_tiles
            t0 = b * T + tch * 512
            xt_t = xtp.tile([128, NCC, 512], wdt, tag="xt")
            for cc in range(NCC):
                nc.sync.dma_start(
                    xt_t[:, cc, :],
                    xT[cc * 128:(cc + 1) * 128, t0:t0 + 512] if MM_DT == "fp16"
                    else xT[cc * 128:(cc + 1) * 128, t0:t0 + 512].bitcast(f32r))
            for jj in range(6):  # q_h0, q_h1, k_h0, k_h1, v_h0, v_h1
                qk_ps = psA.tile([128, 512], f32, tag="psA")
                for cc in range(NCC):
                    nc.tensor.matmul(qk_ps, w_sb[:, cc, jj, :], xt_t[:, cc, :],
                                     start=(cc == 0), stop=(cc == NCC - 1))
                dst = (qt, qt, kt, kt, vt, vt)[jj]
                nc.scalar.copy(dst[:, jj % 2, tch * 512:(tch + 1) * 512], qk_ps)
            # transpose this chunk's V^T slice -> V [t, d]
            for hh in range(HPC):
                for tb in range(4):
                    tg = tch * 4 + tb
                    vp = psT.tile([128, 128], f16, tag="psT")
                    nc.tensor.transpose(
                        vp, vt[:, hh, tg * 128:(tg + 1) * 128], ident_h)
                    nc.vector.tensor_copy(v[:, tg, hh * HD:(hh + 1) * HD], vp)

        def emit_attn_unit(b, qg, h, qkv_tiles, yt):
            qt, kt, vt, v = qkv_tiles
            pt_sb = ptp.tile([128, T // 128, 512], f16, tag="pt")
            den_ps = psD.tile([1, 512], f32, tag="psD")
            yt_ps = psV.tile([128, 512], f32, tag="psV")
            nkb = 4 * qg + 4
            for kb in range(nkb):
                kk = kb - 4 * qg
                qs = max(0, kk) * 128
                st = psA.tile([128, 512], f32, tag="psA")
                nc.tensor.matmul(
                    st[:, qs:512], kt[:, h, kb * 128:(kb + 1) * 128],
                    qt[:, h, qg * 512 + qs:(qg + 1) * 512],
                    start=True, stop=True)
                if kk >= 0:
                    nc.vector.tensor_add(
                        st[:, qs:qs + 128], st[:, qs:qs + 128], triT)
                nc.scalar.activation(
                    pt_sb[:, kb, qs:512], st[:, qs:512], Exp, scale=SCALE)
                nc.tensor.matmul(
                    den_ps[0:1, qs:512], ones_col, pt_sb[:, kb, qs:512],
                    start=(kb == 0), stop=(kb == nkb - 1))
                nc.tensor.matmul(
                    yt_ps[:, qs:512], v[:, kb, h * HD:(h + 1) * HD],
                    pt_sb[:, kb, qs:512],
                    start=(kb == 0), stop=(kb == nkb - 1))
            rec_row = dnp.tile([1, 512], f32, tag="rec")
            nc.vector.reciprocal(rec_row, den_ps[0:1, :])
            rec16 = dnp.tile([1, 512], f16, tag="rec16")
            nc.scalar.copy(rec16, rec_row)
            r_ps = psD.tile([128, 512], f32, tag="psD")
            nc.tensor.matmul(r_ps, ones_row, rec16, start=True, stop=True)
            r_sb = rp.tile([128, 512], f32, tag="rsb")
            nc.vector.tensor_copy(r_sb, r_ps)
            nc.vector.tensor_mul(yt[:, h, :], yt_ps, r_sb)

        def emit_proj(b, qg, yt):
            for tt in range(4):
                for co in range(4):
                    o_ps = psA.tile([128, 512], f32, tag="psA")
                    for jh in range(HPC):
                        nc.tensor.matmul(
                            o_ps, yt[:, jh, tt * 128:(tt + 1) * 128],
                            wp_sb[:, jh, co * 512:(co + 1) * 512],
                            start=(jh == 0), stop=(jh == HPC - 1))
                    o_sb = op.tile([128, 512], f32, tag="osb")
                    nc.vector.tensor_copy(o_sb, o_ps)
                    r0 = b * T + qg * 512 + tt * 128
                    nc.sync.dma_start(
                        out[r0:r0 + 128, co * 512:(co + 1) * 512], o_sb)

        def alloc_qkv_tiles():
            qt = qkvp.tile([128, HPC, T], wdt, tag="qt")
            kt = qkvp.tile([128, HPC, T], wdt, tag="kt")
            vt = qkvp.tile([128, HPC, T], f16, tag="vt")
            v = qkvp.tile([128, T // 128, HPC * HD], f16, tag="v")
            return (qt, kt, vt, v)

        # Pipeline: QKV chunks of batch b+1 are interleaved into the
        # attention/proj stream of batch b so the PE array never idles
        # long enough for HAM to re-throttle it.
        tiles = alloc_qkv_tiles()
        for tch in range(NTCH):
            emit_qkv_chunk(0, tch, tiles)
        for b in range(B):
            nxt = alloc_qkv_tiles() if b + 1 < B else None
            for qg in range(4):
                if nxt is not None:
                    emit_qkv_chunk(b + 1, qg, nxt)
                yt = ytp.tile([128, HPC, 512], wdt, tag="yt")
                for h in range(HPC):
                    emit_attn_unit(b, qg, h, tiles, yt)
                emit_proj(b, qg, yt)
            tiles = nxt

    nc.compile()
    return nc


def _get_nc():
    if "nc" not in _CACHE:
        _CACHE["nc"] = _build_nc()
    return _CACHE["nc"]


def _make_in_maps(x2d, Wqkv, Wproj):
    hdt = np.float16 if MM_DT == "fp16" else np.float32
    xT = np.ascontiguousarray(x2d.T).astype(hdt)  # [C, B*T]
    in_maps = []
    for c in range(N_CORES):
        h0 = c * HPC
        cols = []
        for part in range(3):  # q, k, v blocks of Wqkv columns
            for h in range(HPC):
                j0 = part * C + (h0 + h) * HD
                cols.append(Wqkv[:, j0:j0 + HD])
        wq = np.ascontiguousarray(np.concatenate(cols, axis=1)).astype(hdt)
        wp = np.ascontiguousarray(Wproj[h0 * HD:(h0 + HPC) * HD, :]).astype(hdt)
        in_maps.append({"xt": xT, "wqkv": wq, "wproj": wp})
    return in_maps


def run_shards(in_maps, trace=False):
    from concourse.bass_utils import run_bass_kernel_spmd
    nc = _get_nc()
    last_err = None
    for _attempt in range(3):
        try:
            return run_bass_kernel_spmd(
                nc, in_maps, core_ids=list(range(N_CORES)), trace=trace)
        except Exception as e:  # transient NRT device errors — retry
            last_err = e
            if "UNAVAILABLE" not in str(e) and "UNRECOVERABLE" not in str(e):
                raise
    raise last_err


def kernel(x, Wqkv, Wproj):
    x = np.asarray(x, dtype=np.float32)
    Wqkv = np.asarray(Wqkv, dtype=np.float32)
    Wproj = np.asarray(Wproj, dtype=np.float32)
    x2d = np.ascontiguousarray(x.reshape(B * T, C))

    in_maps = _make_in_maps(x2d, Wqkv, Wproj)
    res = run_shards(in_maps)

    acc = res.results[0]["out"].astype(np.float64)
    for c in range(1, N_CORES):
        acc += res.results[c]["out"]
    return acc.reshape(B, T, C).astype(np.float32)



# revision 3
# speedup vs baseline: 1.2836x; 1.2836x over previous
"""Causal self-attention (B=4, T=2048, C=2048, H=16) on 8 trn2 NeuronCores.

Sharding: tensor-parallel over heads — 2 heads per core. Every core gets the
full (pre-transposed) activation xT, its 2 heads' slice of Wqkv columns and
Wproj rows, computes a full [B*T, C] partial output, and the host sums the 8
partials (the "all-reduce after output projection" done host-side).

v2 dataflow (all matmuls fp16 on PE; softmax tail is PE-free):
  xT tiles --DMA--> QKV proj -> Q^T,K^T [d,t] + V [t,d] (fp16)
  S^T = K^T-block.T @ Q^T chunks (PSUM f32) -> +causal mask (DVE)
  exp (ACT) -> P^T (fp16) -> DVE-accumulated denominator (acc over k-blocks)
  y^T = sum_k V_k^T-block @ P^T-block (PSUM f32)
  den all-partition sum via gpsimd.partition_all_reduce -> DVE fast reciprocal
  yt = y^T * (1/den)  (DVE) -> proj: out_partial = yt.T @ Wproj-rows -> DMA

Scheduling: the PE executes its queue in order, so the emission order IS the
schedule.  Each (b, qg) block interleaves the serial attention steps (score ->
exp -> yt, latency-bound on ACT) with independent "filler" matmul quanta: the
next t-chunk's QKV projection and the previous block's output projection.
The softmax tail (all_reduce + reciprocal + normalize) runs entirely on
gpsimd/DVE, so it never stalls the PE queue.
"""
import numpy as np

B, T, C = 4, 2048, 2048
H, HD = 16, 128
N_CORES = 8
HPC = H // N_CORES          # heads per core = 2
SCALE = float(1.0 / np.sqrt(HD))
NEG = -1e9
MM_DT = "fp16"

_CACHE = {}


def _build_nc():
    import concourse.bass as bass
    from concourse import bacc
    import concourse.tile as tile
    import concourse.mybir as mybir
    import concourse.bass_isa as bass_isa
    from concourse.masks import make_identity
    from contextlib import ExitStack

    f32 = mybir.dt.float32
    f16 = mybir.dt.float16
    Exp = mybir.ActivationFunctionType.Exp

    nc = bacc.Bacc("TRN2", target_bir_lowering=False, debug=False,
                   enable_asserts=True, num_devices=N_CORES)

    xT = nc.dram_tensor("xt", [C, B * T], f16, kind="ExternalInput").ap()
    wqkv = nc.dram_tensor("wqkv", [C, 6 * HD], f16, kind="ExternalInput").ap()
    wproj = nc.dram_tensor("wproj", [HPC * HD, C], f16, kind="ExternalInput").ap()
    out = nc.dram_tensor("out", [B * T, C], f32, kind="ExternalOutput").ap()

    wqkv_v = wqkv.rearrange("(cc p) (jj d) -> p cc jj d", p=128, d=HD)  # [128,16,6,128]
    wproj_v = wproj.rearrange("(jh p) c -> p jh c", p=128)              # [128,2,2048]
    xT_v = xT.rearrange("(cc p) t -> p cc t", p=128)                    # [128,16,B*T]

    NCC = C // 128        # 16 contraction chunks
    NTCH = T // 512       # 4 t-chunks per batch

    with tile.TileContext(nc) as tc, ExitStack() as ctx:
        const = ctx.enter_context(tc.tile_pool(name="const", bufs=1))
        wpool = ctx.enter_context(tc.tile_pool(name="w", bufs=1))
        xtp = ctx.enter_context(tc.tile_pool(name="xt", bufs=2))
        qkvp = ctx.enter_context(tc.tile_pool(name="qkv", bufs=2))
        ptp = ctx.enter_context(tc.tile_pool(name="pt", bufs=6))
        accp = ctx.enter_context(tc.tile_pool(name="acc", bufs=2))
        rp = ctx.enter_context(tc.tile_pool(name="r", bufs=4))
        ytp = ctx.enter_context(tc.tile_pool(name="yt", bufs=2))
        op = ctx.enter_context(tc.tile_pool(name="o", bufs=6))
        psA = ctx.enter_context(tc.tile_pool(name="psA", bufs=3, space="PSUM"))
        psS = ctx.enter_context(tc.tile_pool(name="psS", bufs=3, space="PSUM"))
        psV = ctx.enter_context(tc.tile_pool(name="psV", bufs=2, space="PSUM"))

        ident_f = const.tile([128, 128], f32)
        make_identity(nc, ident_f)
        ident_h = const.tile([128, 128], f16)
        nc.scalar.copy(ident_h, ident_f)
        # transposed-orientation causal mask: keep (partition=k_rel) <= (free=q_rel)
        triT = const.tile([128, 128], f32)
        nc.gpsimd.memset(triT, 0.0)
        nc.gpsimd.affine_select(
            out=triT, in_=triT, compare_op=mybir.AluOpType.is_ge, fill=NEG,
            base=0, pattern=[[1, 128]], channel_multiplier=-1)

        w_sb = wpool.tile([128, NCC, 6, HD], f16)
        for g in range(4):  # split so the first QKV matmul starts early
            nc.sync.dma_start(w_sb[:, 4 * g:4 * g + 4], wqkv_v[:, 4 * g:4 * g + 4])
        wp_sb = wpool.tile([128, 2, C], f16)
        nc.sync.dma_start(wp_sb, wproj_v)

        def chunk_quanta(b, tch, tiles):
            """QKV projection of one 512-token chunk, as a list of small
            emission quanta (fillers for the PE between attention steps)."""
            qt, kt, vt, v = tiles
            t0 = b * T + tch * 512
            xt_t = xtp.tile([128, NCC, 512], f16, tag="xt")
            quanta = []

            def dma_thunk():
                nc.sync.dma_start(xt_t, xT_v[:, :, t0:t0 + 512])
            quanta.append(dma_thunk)

            state = {}

            def mm_thunk(jj, ccg):
                def run():
                    if ccg == 0:
                        state[jj] = psA.tile([128, 512], f32, tag="psA", name="qkps")
                    qk_ps = state[jj]
                    for cc in range(4 * ccg, 4 * ccg + 4):
                        nc.tensor.matmul(qk_ps, w_sb[:, cc, jj, :],
                                         xt_t[:, cc, :],
                                         start=(cc == 0), stop=(cc == NCC - 1))
                    if ccg == 3:
                        dst = (qt, qt, kt, kt, vt, vt)[jj]
                        nc.scalar.copy(
                            dst[:, jj % 2, tch * 512:(tch + 1) * 512], qk_ps)
                return run
            for jj in range(6):
                for ccg in range(4):
                    quanta.append(mm_thunk(jj, ccg))

            def tr_thunk(hh, tb):
                def run():
                    tg = tch * 4 + tb
                    vp = psA.tile([128, 128], f16, tag="psA")
                    nc.tensor.transpose(
                        vp, vt[:, hh, tg * 128:(tg + 1) * 128], ident_h)
                    nc.vector.tensor_copy(v[:, tg, hh * HD:(hh + 1) * HD], vp)
                return run
            for hh in range(HPC):
                for tb in range(4):
                    quanta.append(tr_thunk(hh, tb))
            return quanta

        def proj_quanta(b, qg, yt):
            quanta = []

            def pr_thunk(tt, co):
                def run():
                    o_ps = psA.tile([128, 512], f32, tag="psA")
                    for jh in range(HPC):
                        nc.tensor.matmul(
                            o_ps, yt[:, jh, tt * 128:(tt + 1) * 128],
                            wp_sb[:, jh, co * 512:(co + 1) * 512],
                            start=(jh == 0), stop=(jh == HPC - 1))
                    o_sb = op.tile([128, 512], f32, tag="osb")
                    nc.vector.tensor_copy(o_sb, o_ps)
                    r0 = b * T + qg * 512 + tt * 128
                    nc.sync.dma_start(
                        out[r0:r0 + 128, co * 512:(co + 1) * 512], o_sb)
                return run
            for tt in range(4):
                for co in range(4):
                    quanta.append(pr_thunk(tt, co))
            return quanta

        def unit_steps(b, qg, tiles, yt):
            """Attention for both heads of one 512-query group, h-interleaved.
            Softmax denominator accumulates on DVE; tail is PE-free."""
            qt, kt, vt, v = tiles
            nkb = 4 * qg + 4
            acc = [None, None]
            yt_ps = [None, None]

            def step(h, kb):
                def run():
                    kk = kb - 4 * qg
                    qs = max(0, kk) * 128
                    q0 = qg * 512
                    st = psS.tile([128, 512], f32, tag="st")
                    nc.tensor.matmul(
                        st[:, qs:512], kt[:, h, kb * 128:(kb + 1) * 128],
                        qt[:, h, q0 + qs:q0 + 512], start=True, stop=True)
                    if kk >= 0:
                        nc.vector.tensor_add(
                            st[:, qs:qs + 128], st[:, qs:qs + 128], triT)
                    pt = ptp.tile([128, 512], f16, tag="pt")
                    nc.scalar.activation(
                        pt[:, qs:512], st[:, qs:512], Exp, scale=SCALE)
                    if kb == 0:
                        acc[h] = accp.tile([128, 512], f16, tag="acc", name="accd")
                        yt_ps[h] = psV.tile([128, 512], f32, tag="psV", name="ytps")
                        nc.vector.tensor_copy(acc[h], pt)
                    else:
                        nc.vector.tensor_add(
                            acc[h][:, qs:512], acc[h][:, qs:512], pt[:, qs:512])
                    nc.tensor.matmul(
                        yt_ps[h][:, qs:512], v[:, kb, h * HD:(h + 1) * HD],
                        pt[:, qs:512], start=(kb == 0), stop=(kb == nkb - 1))
                return run

            def tail(h):
                def run():
                    r_all = rp.tile([128, 512], f32, tag="r")
                    nc.gpsimd.partition_all_reduce(
                        r_all, acc[h], 128, bass_isa.ReduceOp.add)
                    rec = rp.tile([128, 512], f32, tag="rec")
                    nc.vector.reciprocal_approx_fast(rec, r_all)
                    nc.vector.tensor_mul(yt[:, h, :], yt_ps[h], rec)
                return run

            steps = []
            for kb in range(nkb):
                for h in range(HPC):
                    steps.append(step(h, kb))
            steps.append(tail(0))
            steps.append(tail(1))
            return steps

        def alloc_qkv_tiles():
            qt = qkvp.tile([128, HPC, T], f16, tag="qt")
            kt = qkvp.tile([128, HPC, T], f16, tag="kt")
            vt = qkvp.tile([128, HPC, T], f16, tag="vt")
            v = qkvp.tile([128, T // 128, HPC * HD], f16, tag="v")
            return (qt, kt, vt, v)

        chunks = [(b, t) for b in range(B) for t in range(NTCH)]
        tiles_cur = alloc_qkv_tiles()
        tiles_nxt = None
        for q in chunk_quanta(*chunks[0], tiles_cur):  # warmup: chunk (0,0)
            q()
        ci = 1
        pending_proj = None
        for b in range(B):
            for qg in range(NTCH):
                fillers = []
                if ci < len(chunks):
                    cb, ct = chunks[ci]
                    ci += 1
                    if cb != b:
                        tiles_nxt = alloc_qkv_tiles()
                    fillers += chunk_quanta(
                        cb, ct, tiles_cur if cb == b else tiles_nxt)
                if pending_proj is not None:
                    fillers += proj_quanta(*pending_proj)
                yt = ytp.tile([128, HPC, 512], f16, tag="yt")
                steps = unit_steps(b, qg, tiles_cur, yt)
                nf, ns = len(fillers), len(steps)
                fi = min(2, nf)
                for q in fillers[:fi]:  # prime the PE (incl. the xt DMA)
                    q()
                for si, s in enumerate(steps):
                    s()
                    tgt = min(2 + (si + 1) * (nf - 2) // ns, nf) if nf > 2 else fi
                    while fi < tgt:
                        fillers[fi]()
                        fi += 1
                while fi < nf:
                    fillers[fi]()
                    fi += 1
                pending_proj = (b, qg, yt)
            if tiles_nxt is not None:
                tiles_cur, tiles_nxt = tiles_nxt, None
        for q in proj_quanta(*pending_proj):
            q()

    nc.compile()
    return nc


def _get_nc():
    if "nc" not in _CACHE:
        _CACHE["nc"] = _build_nc()
    return _CACHE["nc"]


def _make_in_maps(x2d, Wqkv, Wproj):
    hdt = np.float16
    xT = np.ascontiguousarray(x2d.T).astype(hdt)  # [C, B*T]
    in_maps = []
    for c in range(N_CORES):
        h0 = c * HPC
        cols = []
        for part in range(3):  # q, k, v blocks of Wqkv columns
            for h in range(HPC):
                j0 = part * C + (h0 + h) * HD
                cols.append(Wqkv[:, j0:j0 + HD])
        wq = np.ascontiguousarray(np.concatenate(cols, axis=1)).astype(hdt)
        wp = np.ascontiguousarray(Wproj[h0 * HD:(h0 + HPC) * HD, :]).astype(hdt)
        in_maps.append({"xt": xT, "wqkv": wq, "wproj": wp})
    return in_maps


def run_shards(in_maps, trace=False):
    from concourse.bass_utils import run_bass_kernel_spmd
    nc = _get_nc()
    last_err = None
    for _attempt in range(3):
        try:
            return run_bass_kernel_spmd(
                nc, in_maps, core_ids=list(range(N_CORES)), trace=trace)
        except Exception as e:  # transient NRT device errors — retry
            last_err = e
            if "UNAVAILABLE" not in str(e) and "UNRECOVERABLE" not in str(e):
                raise
    raise last_err


def kernel(x, Wqkv, Wproj):
    x = np.asarray(x, dtype=np.float32)
    Wqkv = np.asarray(Wqkv, dtype=np.float32)
    Wproj = np.asarray(Wproj, dtype=np.float32)
    x2d = np.ascontiguousarray(x.reshape(B * T, C))

    in_maps = _make_in_maps(x2d, Wqkv, Wproj)
    res = run_shards(in_maps)

    acc = res.results[0]["out"].astype(np.float64)
    for c in range(1, N_CORES):
        acc += res.results[c]["out"]
    return acc.reshape(B, T, C).astype(np.float32)


# revision 8
# speedup vs baseline: 1.3347x; 1.0398x over previous
"""Causal self-attention (B=4, T=2048, C=2048, H=16) on 8 trn2 NeuronCores.

Sharding: tensor-parallel over heads — 2 heads per core. Every core gets the
full (pre-transposed) activation xT, its 2 heads' slice of Wqkv columns and
Wproj rows, computes a full [B*T, C] partial output, and the host sums the 8
partials (the "all-reduce after output projection" done host-side).

v2 dataflow (all matmuls fp16 on PE; softmax tail is PE-free):
  xT tiles --DMA--> QKV proj -> Q^T,K^T [d,t] + V [t,d] (fp16)
  S^T = K^T-block.T @ Q^T chunks (PSUM f32) -> +causal mask (DVE)
  exp (ACT) -> P^T (fp16) -> DVE-accumulated denominator (acc over k-blocks)
  y^T = sum_k V_k^T-block @ P^T-block (PSUM f32)
  den all-partition sum via gpsimd.partition_all_reduce -> DVE fast reciprocal
  yt = y^T * (1/den)  (DVE) -> proj: out_partial = yt.T @ Wproj-rows -> DMA

Scheduling: the PE executes its queue in order, so the emission order IS the
schedule.  Each (b, qg) block interleaves the serial attention steps (score ->
exp -> yt, latency-bound on ACT) with independent "filler" matmul quanta: the
next t-chunk's QKV projection and the previous block's output projection.
The softmax tail (all_reduce + reciprocal + normalize) runs entirely on
gpsimd/DVE, so it never stalls the PE queue.
"""
import numpy as np

B, T, C = 4, 2048, 2048
H, HD = 16, 128
N_CORES = 8
HPC = H // N_CORES          # heads per core = 2
SCALE = float(1.0 / np.sqrt(HD))
NEG = -1e9
MM_DT = "fp16"

_CACHE = {}


def _build_nc():
    import concourse.bass as bass
    from concourse import bacc
    import concourse.tile as tile
    import concourse.mybir as mybir
    import concourse.bass_isa as bass_isa
    from concourse.masks import make_identity
    from contextlib import ExitStack

    f32 = mybir.dt.float32
    f16 = mybir.dt.float16
    Exp = mybir.ActivationFunctionType.Exp

    nc = bacc.Bacc("TRN2", target_bir_lowering=False, debug=False,
                   enable_asserts=True, num_devices=N_CORES)

    xT = nc.dram_tensor("xt", [C, B * T], f16, kind="ExternalInput").ap()
    wqkv = nc.dram_tensor("wqkv", [C, 6 * HD], f16, kind="ExternalInput").ap()
    wproj = nc.dram_tensor("wproj", [HPC * HD, C], f16, kind="ExternalInput").ap()
    out = nc.dram_tensor("out", [B * T, C], f32, kind="ExternalOutput").ap()

    wqkv_v = wqkv.rearrange("(cc p) (jj d) -> p cc jj d", p=128, d=HD)  # [128,16,6,128]
    wproj_v = wproj.rearrange("(jh p) c -> p jh c", p=128)              # [128,2,2048]
    xT_v = xT.rearrange("(cc p) t -> p cc t", p=128)                    # [128,16,B*T]

    NCC = C // 128        # 16 contraction chunks
    NTCH = T // 512       # 4 t-chunks per batch

    with tile.TileContext(nc) as tc, ExitStack() as ctx:
        const = ctx.enter_context(tc.tile_pool(name="const", bufs=1))
        wpool = ctx.enter_context(tc.tile_pool(name="w", bufs=1))
        xtp = ctx.enter_context(tc.tile_pool(name="xt", bufs=2))
        qkvp = ctx.enter_context(tc.tile_pool(name="qkv", bufs=2))
        ptp = ctx.enter_context(tc.tile_pool(name="pt", bufs=6))
        accp = ctx.enter_context(tc.tile_pool(name="acc", bufs=2))
        rp = ctx.enter_context(tc.tile_pool(name="r", bufs=4))
        ysbp = ctx.enter_context(tc.tile_pool(name="ysb", bufs=3))
        ytp = ctx.enter_context(tc.tile_pool(name="yt", bufs=2))
        op = ctx.enter_context(tc.tile_pool(name="o", bufs=6))
        psA = ctx.enter_context(tc.tile_pool(name="psA", bufs=3, space="PSUM"))
        psS = ctx.enter_context(tc.tile_pool(name="psS", bufs=3, space="PSUM"))
        psV = ctx.enter_context(tc.tile_pool(name="psV", bufs=2, space="PSUM"))

        ident_f = const.tile([128, 128], f32)
        make_identity(nc, ident_f)
        ident_h = const.tile([128, 128], f16)
        nc.scalar.copy(ident_h, ident_f)
        # transposed-orientation causal mask: keep (partition=k_rel) <= (free=q_rel)
        triT = const.tile([128, 128], f32)
        nc.gpsimd.memset(triT, 0.0)
        nc.gpsimd.affine_select(
            out=triT, in_=triT, compare_op=mybir.AluOpType.is_ge, fill=NEG,
            base=0, pattern=[[1, 128]], channel_multiplier=-1)
        ones_col = const.tile([128, 1], f16)
        nc.vector.memset(ones_col, 1.0)
        ones_row = const.tile([1, 128], f16)
        nc.vector.memset(ones_row, 1.0)

        w_sb = wpool.tile([128, NCC, 6, HD], f16)
        wp_sb = wpool.tile([128, 2, C], f16)

        def chunk_quanta(b, tch, tiles, split_dma=False):
            """QKV projection of one 512-token chunk, as a list of small
            emission quanta (fillers for the PE between attention steps)."""
            qt, kt, vt, v = tiles
            t0 = b * T + tch * 512
            xt_t = xtp.tile([128, NCC, 512], f16, tag="xt")
            quanta = []

            if split_dma:
                def dma_part(g):
                    def run():
                        nc.sync.dma_start(xt_t[:, 4 * g:4 * g + 4],
                                          xT_v[:, 4 * g:4 * g + 4, t0:t0 + 512])
                    return run
                for g in range(4):
                    quanta.append(dma_part(g))
            else:
                def dma_thunk():
                    nc.sync.dma_start(xt_t, xT_v[:, :, t0:t0 + 512])
                quanta.append(dma_thunk)

            state = {}

            def mm_thunk(jj, ccg):
                def run():
                    if ccg == 0:
                        state[jj] = psA.tile([128, 512], f32, tag="psA", name="qkps")
                    qk_ps = state[jj]
                    for cc in range(4 * ccg, 4 * ccg + 4):
                        nc.tensor.matmul(qk_ps, w_sb[:, cc, jj, :],
                                         xt_t[:, cc, :],
                                         start=(cc == 0), stop=(cc == NCC - 1))
                    if ccg == 3:
                        dst = (qt, qt, kt, kt, vt, vt)[jj]
                        nc.scalar.copy(
                            dst[:, jj % 2, tch * 512:(tch + 1) * 512], qk_ps)
                return run
            for jj in range(6):
                for ccg in range(4):
                    quanta.append(mm_thunk(jj, ccg))

            def tr_thunk(hh, tb):
                def run():
                    tg = tch * 4 + tb
                    vp = psA.tile([128, 128], f16, tag="psA")
                    nc.tensor.transpose(
                        vp, vt[:, hh, tg * 128:(tg + 1) * 128], ident_h)
                    nc.vector.tensor_copy(v[:, tg, hh * HD:(hh + 1) * HD], vp)
                return run
            for hh in range(HPC):
                for tb in range(4):
                    quanta.append(tr_thunk(hh, tb))
            return quanta

        def proj_quanta(b, qg, yt):
            quanta = []

            def pr_thunk(tt, co):
                def run():
                    o_ps = psA.tile([128, 512], f32, tag="psA")
                    for jh in range(HPC):
                        nc.tensor.matmul(
                            o_ps, yt[:, jh, tt * 128:(tt + 1) * 128],
                            wp_sb[:, jh, co * 512:(co + 1) * 512],
                            start=(jh == 0), stop=(jh == HPC - 1))
                    o_sb = op.tile([128, 512], f32, tag="osb")
                    nc.vector.tensor_copy(o_sb, o_ps)
                    r0 = b * T + qg * 512 + tt * 128
                    nc.sync.dma_start(
                        out[r0:r0 + 128, co * 512:(co + 1) * 512], o_sb)
                return run
            for tt in range(4):
                for co in range(4):
                    quanta.append(pr_thunk(tt, co))
            return quanta

        def unit_steps(b, qg, tiles, yt, last=False):
            """Attention for both heads of one 512-query group, h-interleaved.
            Softmax denominator accumulates on DVE; tail is PE-free except in
            the very last block, where a PE den-reduce + broadcast has lower
            latency than the serialized gpsimd all-reduces."""
            qt, kt, vt, v = tiles
            nkb = 4 * qg + 4
            acc = [None, None]
            yt_ps = [None, None]
            ysb = [None, None]

            def step(h, kb):
                def run():
                    kk = kb - 4 * qg
                    qs = max(0, kk) * 128
                    q0 = qg * 512
                    st = psS.tile([128, 512], f32, tag="st")
                    nc.tensor.matmul(
                        st[:, qs:512], kt[:, h, kb * 128:(kb + 1) * 128],
                        qt[:, h, q0 + qs:q0 + 512], start=True, stop=True)
                    if kk >= 0:
                        nc.vector.tensor_add(
                            st[:, qs:qs + 128], st[:, qs:qs + 128], triT)
                    pt = ptp.tile([128, 512], f16, tag="pt")
                    nc.scalar.activation(
                        pt[:, qs:512], st[:, qs:512], Exp, scale=SCALE)
                    if kb == 0:
                        acc[h] = accp.tile([128, 512], f16, tag="acc", name="accd")
                        yt_ps[h] = psV.tile([128, 512], f32, tag="psV", name="ytps")
                        nc.vector.tensor_copy(acc[h], pt)
                    else:
                        nc.vector.tensor_add(
                            acc[h][:, qs:512], acc[h][:, qs:512], pt[:, qs:512])
                    nc.tensor.matmul(
                        yt_ps[h][:, qs:512], v[:, kb, h * HD:(h + 1) * HD],
                        pt[:, qs:512], start=(kb == 0), stop=(kb == nkb - 1))
                return run

            def ycopy(h):
                def run():  # frees the psV bank without waiting on the tail
                    ysb[h] = ysbp.tile([128, 512], f32, tag="ysb", name="ysb")
                    nc.vector.tensor_copy(ysb[h], yt_ps[h])
                return run

            def tail(h):
                def run():
                    r_all = rp.tile([128, 512], f32, tag="r")
                    nc.gpsimd.partition_all_reduce(
                        r_all, acc[h], 128, bass_isa.ReduceOp.add)
                    rec = rp.tile([128, 512], f32, tag="rec")
                    nc.vector.reciprocal_approx_fast(rec, r_all)
                    nc.vector.tensor_mul(yt[:, h, :], ysb[h], rec)
                return run

            def tail_pe(h):
                def run():
                    den_ps = psA.tile([1, 512], f32, tag="psA", name="denp")
                    nc.tensor.matmul(den_ps, ones_col, acc[h],
                                     start=True, stop=True)
                    rec1 = rp.tile([1, 512], f32, tag="rec")
                    nc.vector.reciprocal_approx_fast(rec1, den_ps)
                    rec16 = rp.tile([1, 512], f16, tag="r")
                    nc.scalar.copy(rec16, rec1)
                    r_ps = psA.tile([128, 512], f32, tag="psA", name="rps")
                    nc.tensor.matmul(r_ps, ones_row, rec16,
                                     start=True, stop=True)
                    rsb = rp.tile([128, 512], f32, tag="rsb")
                    nc.vector.tensor_copy(rsb, r_ps)
                    nc.vector.tensor_mul(yt[:, h, :], ysb[h], rsb)
                return run

            steps = []
            for kb in range(nkb):
                for h in range(HPC):
                    steps.append(step(h, kb))
            steps.append(ycopy(0))
            steps.append(ycopy(1))
            t = tail_pe if last else tail
            steps.append(t(0))
            steps.append(t(1))
            return steps

        def alloc_qkv_tiles():
            qt = qkvp.tile([128, HPC, T], f16, tag="qt")
            kt = qkvp.tile([128, HPC, T], f16, tag="kt")
            vt = qkvp.tile([128, HPC, T], f16, tag="vt")
            v = qkvp.tile([128, T // 128, HPC * HD], f16, tag="v")
            return (qt, kt, vt, v)

        chunks = [(b, t) for b in range(B) for t in range(NTCH)]
        tiles_cur = alloc_qkv_tiles()
        tiles_nxt = None
        # Warmup: first w chunk, then the first xt chunk (split so the first
        # QKV matmuls start ~3us in), then the rest of the weights.
        warm = chunk_quanta(*chunks[0], tiles_cur, split_dma=True)
        nc.sync.dma_start(w_sb[:, 0:4], wqkv_v[:, 0:4])
        for q in warm[:4]:  # the 4 xt sub-DMAs
            q()
        for g in range(1, 4):
            nc.sync.dma_start(w_sb[:, 4 * g:4 * g + 4], wqkv_v[:, 4 * g:4 * g + 4])
        nc.sync.dma_start(wp_sb, wproj_v)
        for q in warm[4:]:
            q()
        ci = 1
        pending_proj = None
        for b in range(B):
            for qg in range(NTCH):
                last = (b == B - 1 and qg == NTCH - 1)
                fillers = []
                if ci < len(chunks):
                    cb, ct = chunks[ci]
                    ci += 1
                    if cb != b:
                        tiles_nxt = alloc_qkv_tiles()
                    fillers += chunk_quanta(
                        cb, ct, tiles_cur if cb == b else tiles_nxt)
                if pending_proj is not None:
                    fillers += proj_quanta(*pending_proj)
                yt = ytp.tile([128, HPC, 512], f16, tag="yt")
                steps = unit_steps(b, qg, tiles_cur, yt, last=last)
                reserve = min(6, len(fillers)) if last else 0
                nf, ns = len(fillers) - reserve, len(steps)
                fi = min(2, nf)
                for q in fillers[:fi]:  # prime the PE (incl. the xt DMA)
                    q()
                for si, s in enumerate(steps):
                    s()
                    tgt = min(2 + (si + 1) * (nf - 2) // ns, nf) if nf > 2 else fi
                    while fi < tgt:
                        fillers[fi]()
                        fi += 1
                while fi < nf + reserve:
                    fillers[fi]()
                    fi += 1
                pending_proj = (b, qg, yt)
            if tiles_nxt is not None:
                tiles_cur, tiles_nxt = tiles_nxt, None
        for q in proj_quanta(*pending_proj):
            q()

    nc.compile()
    return nc


def _get_nc():
    if "nc" not in _CACHE:
        _CACHE["nc"] = _build_nc()
    return _CACHE["nc"]


def _make_in_maps(x2d, Wqkv, Wproj):
    hdt = np.float16
    xT = np.ascontiguousarray(x2d.T).astype(hdt)  # [C, B*T]
    in_maps = []
    for c in range(N_CORES):
        h0 = c * HPC
        cols = []
        for part in range(3):  # q, k, v blocks of Wqkv columns
            for h in range(HPC):
                j0 = part * C + (h0 + h) * HD
                cols.append(Wqkv[:, j0:j0 + HD])
        wq = np.ascontiguousarray(np.concatenate(cols, axis=1)).astype(hdt)
        wp = np.ascontiguousarray(Wproj[h0 * HD:(h0 + HPC) * HD, :]).astype(hdt)
        in_maps.append({"xt": xT, "wqkv": wq, "wproj": wp})
    return in_maps


def run_shards(in_maps, trace=False):
    from concourse.bass_utils import run_bass_kernel_spmd
    nc = _get_nc()
    last_err = None
    for _attempt in range(3):
        try:
            return run_bass_kernel_spmd(
                nc, in_maps, core_ids=list(range(N_CORES)), trace=trace)
        except Exception as e:  # transient NRT device errors — retry
            last_err = e
            if "UNAVAILABLE" not in str(e) and "UNRECOVERABLE" not in str(e):
                raise
    raise last_err


def kernel(x, Wqkv, Wproj):
    x = np.asarray(x, dtype=np.float32)
    Wqkv = np.asarray(Wqkv, dtype=np.float32)
    Wproj = np.asarray(Wproj, dtype=np.float32)
    x2d = np.ascontiguousarray(x.reshape(B * T, C))

    in_maps = _make_in_maps(x2d, Wqkv, Wproj)
    res = run_shards(in_maps)

    acc = res.results[0]["out"].astype(np.float64)
    for c in range(1, N_CORES):
        acc += res.results[c]["out"]
    return acc.reshape(B, T, C).astype(np.float32)
